# revision 5
# baseline (speedup 1.0000x reference)
"""Trainium2 Bass kernel for nn_CBAM_83691732730338.

Self-attention block (HWxHW attention over (C,D)-channels) + residual:
  x = transpose(x2d)                        # (B, C, D, H, W)
  q/k/v = 1x1 conv over C (collapsed to channel matmuls, D folded into
          the attention channel dim), N = H*W
  energy = q^T k  (per batch, N x N), attn = softmax(energy, axis=-1)
  out = v @ attn^T ; out = gamma*out + x3d

Sharding: 8 cores = 4 batches x 2 spatial halves (rows n in [half*2048,
half*2048+2048)). Each core computes k/v for its whole batch, q for its
half, the (4096 m x 2048 n) transposed-attention block, and its half of
the output. No cross-core communication.

Kernel-internal layouts (per core):
  xA    [65, 16384]  : x[b] as (c, h*W*D + w*D + d), row 64 = 1.0 (bias trick)
  k_sb  [32, 4096]   : k[(d*8+cq), m]      (d-major channel order)
  q_sb  [32, 2048]   : q[(d*8+cq), n_local]
  vt    [128, 32*257]: chunk-major v^T; cols [ch*257 + d*64 + c] = v[(d,c), m],
                       col ch*257+256 = 1.0 (row-sum trick for softmax denom)
  P_t   [128, 2048]x8 groups per window: exp(energy)[m, n]
  av    psum [128, 257]: cols 0..255 = unnormalized out[n, (d,c)], col 256 = sum_m
  out_A [64, 8192]   : final (c, hw_local*4 + d), preloaded with x3d slice

All matmuls run as float32r (full-rate fp32 PE mode).
"""

import sys
import numpy as np

sys.path.insert(0, "/opt/trn_rl_repo")

C = 64
D = 4
CQ = 8
H = 64
W = 64
N = H * W          # 4096 spatial positions per batch
NH = N // 2        # 2048 per core
KD = D * CQ        # 32  attention contraction channels
CD = D * C         # 256 attention value channels
NCORES = 8

_cache = {}


def _build_program():
    import concourse.bacc as bacc
    import concourse.bass as bass
    import concourse.mybir as mybir
    import concourse.tile as tile
    from contextlib import ExitStack

    F32 = mybir.dt.float32
    F32R = mybir.dt.float32r
    Exp = mybir.ActivationFunctionType.Exp
    ADD = mybir.AluOpType.add

    nc = bacc.Bacc("TRN2", target_bir_lowering=False)

    x_d = nc.dram_tensor("x", [C, N * D], F32R, kind="ExternalInput")
    xq_d = nc.dram_tensor("xq", [C, NH * D], F32R, kind="ExternalInput")
    x3_d = nc.dram_tensor("x3", [C, NH * D], F32, kind="ExternalInput")
    wqb_d = nc.dram_tensor("wqb", [C + 1, 128], F32R, kind="ExternalInput")
    wkb_d = nc.dram_tensor("wkb", [C + 1, 128], F32R, kind="ExternalInput")
    wv_d = nc.dram_tensor("wv", [C + 1, C], F32R, kind="ExternalInput")
    gm_d = nc.dram_tensor("gamma", [1, 1], F32, kind="ExternalInput")
    id_d = nc.dram_tensor("ident", [128, 128], F32, kind="ExternalInput")
    ones_d = nc.dram_tensor("ones", [1, N * D], F32R, kind="ExternalInput")
    out_d = nc.dram_tensor("out", [C, NH * D], F32, kind="ExternalOutput")

    with tile.TileContext(nc) as tc, ExitStack() as ctx:
        consts = ctx.enter_context(tc.tile_pool(name="consts", bufs=1))
        qkv = ctx.enter_context(tc.tile_pool(name="qkv", bufs=1))
        outp = ctx.enter_context(tc.tile_pool(name="outp", bufs=1))

        wqb = consts.tile([C + 1, 128], F32R)
        nc.sync.dma_start(out=wqb, in_=wqb_d[:, :])
        wkb = consts.tile([C + 1, 128], F32R)
        nc.sync.dma_start(out=wkb, in_=wkb_d[:, :])
        wv = consts.tile([C + 1, C], F32R)
        nc.sync.dma_start(out=wv, in_=wv_d[:, :])
        ident = consts.tile([128, 128], F32)
        nc.sync.dma_start(out=ident, in_=id_d[:, :])
        gam = consts.tile([128, 1], F32)
        nc.sync.dma_start(out=gam, in_=gm_d[:, :].partition_broadcast(128))

        k_sb = qkv.tile([KD, N], F32R)
        q_sb = qkv.tile([KD, NH], F32R)
        vt = qkv.tile([128, 32 * 258], F32R)
        out_A = outp.tile([C, NH * D], F32)
        nc.sync.dma_start(out=out_A, in_=x3_d[:, :])

        # ---------------- Phase A: QKV convs ----------------
        xa_stack = ExitStack()
        xapool = xa_stack.enter_context(tc.tile_pool(name="xa", bufs=1))
        xA = xapool.tile([C + 1, N * D], F32R)
        nc.sync.dma_start(out=xA[0:C, :], in_=x_d[:, :])
        nc.sync.dma_start(out=xA[C : C + 1, :], in_=ones_d[:, :])
        xq = xapool.tile([C + 1, NH * D], F32R)
        nc.sync.dma_start(out=xq[0:C, :], in_=xq_d[:, :])
        nc.sync.dma_start(out=xq[C : C + 1, :], in_=ones_d[:, 0 : NH * D])

        xa3 = xA.rearrange("p (n d) -> p n d", d=D)    # [65, 4096, 4]
        xq3 = xq.rearrange("p (n d) -> p n d", d=D)    # [65, 2048, 4]
        vt3 = vt.rearrange("p (ch q) -> p ch q", q=258)  # [128, 32, 258]

        psA_stack = ExitStack()
        psA = psA_stack.enter_context(
            tc.tile_pool(name="psA", space="PSUM", bufs=2)
        )
        for w in range(8):
            k_ps = psA.tile([KD, 512], F32, tag="kq_ps")
            for d in range(D):
                nc.tensor.matmul(
                    k_ps,
                    wkb[:, d * 32 : (d + 1) * 32],
                    xa3[:, w * 512 : (w + 1) * 512, d],
                    start=(d == 0),
                    stop=(d == D - 1),
                )
            nc.vector.tensor_copy(out=k_sb[:, w * 512 : (w + 1) * 512], in_=k_ps)
        for w in range(4):
            q_ps = psA.tile([KD, 512], F32, tag="kq_ps")
            for d in range(D):
                nc.tensor.matmul(
                    q_ps,
                    wqb[:, d * 32 : (d + 1) * 32],
                    xq3[:, w * 512 : (w + 1) * 512, d],
                    start=(d == 0),
                    stop=(d == D - 1),
                )
            nc.vector.tensor_copy(out=q_sb[:, w * 512 : (w + 1) * 512], in_=q_ps)
        for d in range(D):
            for g in range(8):
                v_ps = psA.tile([128, 256], F32, tag="v_ps")
                for cc in range(4):
                    ch = g * 4 + cc
                    nc.tensor.matmul(
                        v_ps[:, cc * C : (cc + 1) * C],
                        xa3[:, ch * 128 : (ch + 1) * 128, d],
                        wv[:, :],
                        start=True,
                        stop=True,
                    )
                nc.vector.tensor_copy(
                    out=vt3[:, g * 4 : (g + 1) * 4, d * C : (d + 1) * C],
                    in_=v_ps.rearrange("p (cc o) -> p cc o", o=C),
                )
        nc.sync.dma_start(
            out=vt3[:, :, 256:258],
            in_=bass.AP(ones_d, 0, [[0, 128], [1, 32], [1, 2]]),
        )
        psA_stack.close()
        xa_stack.close()

        # ---------------- Phase B: attention ----------------
        ptpool = ctx.enter_context(tc.tile_pool(name="pt", bufs=10))
        work = ctx.enter_context(tc.tile_pool(name="work", bufs=3))
        sm = ctx.enter_context(tc.tile_pool(name="sm", bufs=4))
        psE = ctx.enter_context(tc.tile_pool(name="psE", space="PSUM", bufs=2))
        psAV = ctx.enter_context(tc.tile_pool(name="psAV", space="PSUM", bufs=2))
        psT = ctx.enter_context(tc.tile_pool(name="psT", space="PSUM", bufs=2))

        def emit_et_group(wi, g):
            """E_t + exp for m-chunks 4g..4g+3 of window wi -> one P_t group."""
            ptg = ptpool.tile([128, 2048], F32R, tag="ptg", name=f"ptg_{wi}_{g}")
            for hv in range(2):
                et = psE.tile([128, 1024], F32, tag="et", name=f"et_{wi}_{g}_{hv}")
                for j in range(2):
                    ch = g * 4 + hv * 2 + j
                    nc.tensor.matmul(
                        et[:, j * 512 : (j + 1) * 512],
                        k_sb[:, ch * 128 : (ch + 1) * 128],
                        q_sb[:, wi * 512 : (wi + 1) * 512],
                        start=True,
                        stop=True,
                    )
                nc.scalar.activation(
                    out=ptg[:, hv * 1024 : (hv + 1) * 1024], in_=et, func=Exp
                )
            return ptg

        def emit_av_block(wi, nb, groups):
            """attn @ [v|1] for n-block nb of window wi, normalize, transpose,
            accumulate into out_A (which holds x3d)."""
            av = psAV.tile([128, 258], F32, tag="av", name=f"av_{wi}_{nb}")
            for ch in range(32):
                g, o = divmod(ch, 4)
                nc.tensor.matmul(
                    av,
                    groups[g][:, o * 512 + nb * 128 : o * 512 + nb * 128 + 128]
                    ,
                    vt[:, ch * 258 : (ch + 1) * 258],
                    start=(ch == 0),
                    stop=(ch == 31),
                )
            rc = sm.tile([128, 1], F32, tag="rc", name=f"rc_{wi}_{nb}")
            nc.vector.reciprocal(rc, av[:, 256:257])
            grc = sm.tile([128, 1], F32, tag="grc", name=f"grc_{wi}_{nb}")
            nc.vector.tensor_scalar_mul(grc, rc, gam)
            osb = work.tile([128, 256], F32, tag="osb", name=f"osb_{wi}_{nb}")
            nc.vector.tensor_scalar_mul(osb, av[:, 0:256], grc)
            tr = psT.tile([64, 512], F32, tag="tr", name=f"tr_{wi}_{nb}")
            for d in range(D):
                nc.tensor.transpose(
                    tr[:, d * 128 : (d + 1) * 128],
                    osb[:, d * C : (d + 1) * C],
                    ident,
                )
            hw0 = wi * 512 + nb * 128
            oslice = (
                out_A.rearrange("p (hw d) -> p hw d", d=D)[:, hw0 : hw0 + 128, :]
                .transpose([0, 2, 1])
            )  # [64, 4, 128] iterated (d, hw) to match tr
            tr3 = tr.rearrange("p (d nn) -> p d nn", nn=128)
            nc.vector.tensor_tensor(out=oslice, in0=tr3, in1=oslice, op=ADD)

        # software pipeline: E_t groups of window w interleave with AV of w-1
        prev_groups = None
        for w in range(4):
            groups = []
            for g in range(8):
                groups.append(emit_et_group(w, g))
                if prev_groups is not None and g % 2 == 1:
                    emit_av_block(w - 1, g // 2, prev_groups)
            prev_groups = groups
        for nb in range(4):
            emit_av_block(3, nb, prev_groups)

        nc.sync.dma_start(out=out_d[:, :], in_=out_A)

    nc.compile()
    return nc


def _get_program():
    if "nc" not in _cache:
        _cache["nc"] = _build_program()
    return _cache["nc"]


def _host_weights(Wq, bq, Wk, bk, Wv, bv):
    """Block-diagonal qk conv weights: lhsT slice [:, d*32:(d+1)*32] maps
    x_aug (65 rows: 64 channels + ones) to psum rows (d*8+cq), zero rows
    for other d (accumulated over the 4 d-matmuls)."""
    wqb = np.zeros((C + 1, 128), np.float32)
    wkb = np.zeros((C + 1, 128), np.float32)
    for d in range(D):
        for cq in range(CQ):
            col = d * 32 + d * CQ + cq
            wqb[0:C, col] = Wq[cq, :]
            wqb[C, col] = bq[cq]
            wkb[0:C, col] = Wk[cq, :]
            wkb[C, col] = bk[cq]
    wv_aug = np.concatenate([Wv.T, bv[None, :]], axis=0).astype(np.float32)
    return wqb, wkb, np.ascontiguousarray(wv_aug)


def _run(inputs, trace=False):
    from concourse.bass_utils import run_bass_kernel_spmd

    x2d = np.asarray(inputs["x2d"], np.float32)
    x3d = np.asarray(inputs["x3d"], np.float32)
    wqb, wkb, wv_aug = _host_weights(
        np.asarray(inputs["Wq"], np.float32), np.asarray(inputs["bq"], np.float32),
        np.asarray(inputs["Wk"], np.float32), np.asarray(inputs["bk"], np.float32),
        np.asarray(inputs["Wv"], np.float32), np.asarray(inputs["bv"], np.float32),
    )
    gamma = np.asarray(inputs["gamma"], np.float32).reshape(1, 1)
    ident = np.eye(128, dtype=np.float32)

    in_maps = []
    for core in range(NCORES):
        b, half = divmod(core, 2)
        xb = np.ascontiguousarray(x2d[b].reshape(C, N * D))
        lo, hi = half * NH * D, (half + 1) * NH * D
        in_maps.append({
            "x": xb,
            "xq": np.ascontiguousarray(xb[:, lo:hi]),
            "x3": np.ascontiguousarray(x3d[b].reshape(C, N * D)[:, lo:hi]),
            "wqb": wqb,
            "wkb": wkb,
            "wv": wv_aug,
            "gamma": gamma,
            "ident": ident,
            "ones": np.ones((1, N * D), np.float32),
        })

    nc = _get_program()
    res = run_bass_kernel_spmd(
        nc, in_maps, core_ids=list(range(NCORES)), trace=trace
    )

    out_full = np.empty((4, C, H, W, D), np.float32)
    for core in range(NCORES):
        b, half = divmod(core, 2)
        o = res.results[core]["out"].reshape(C, H // 2, W, D)
        out_full[b, :, half * (H // 2) : (half + 1) * (H // 2), :, :] = o
    return out_full, res


def kernel(**inputs):
    out, _ = _run(inputs, trace=False)
    return out


# revision 7
# speedup vs baseline: 1.5281x; 1.5281x over previous
"""Trainium2 Bass kernel for nn_CBAM_83691732730338.

Self-attention block (HWxHW attention over (C,D)-channels) + residual:
  x = transpose(x2d)                        # (B, C, D, H, W)
  q/k/v = 1x1 conv over C (collapsed to channel matmuls, D folded into
          the attention channel dim), N = H*W
  energy = q^T k  (per batch, N x N), attn = softmax(energy, axis=-1)
  out = v @ attn^T ; out = gamma*out + x3d

Sharding: 8 cores = 4 batches x 2 spatial halves. Attention is invariant
to a permutation of the softmax/value positions m, so each core receives
its batch's x ROTATED so that the core's n-half sits at positions
0..2047: q is computed from positions 0..2047, k/v over all 4096, and
the program is identical on every core (SPMD) with no runtime offsets.

Kernel-internal layouts (per core):
  xA    [65, 16384]  : rot(x[b]) as (c, hw*D + d), row 64 = 1.0 (bias trick)
  k_sb  [128, 4096]  : k[(d*8+cq), m] replicated 4x along partitions
                       (row r*32 + dq) -- feeds 4x row-tiled energy matmuls
  q_sb  [128, 2048]  : q likewise, n = local 0..2047
  vt    [128, 32*258]: chunk-major v^T; cols [ch*258 + d*64 + c] = v[(d,c), m],
                       cols ch*258+{256,257} = 1.0 (row-sum trick + even pad)
  P_t   [128, 2048]x8 groups per window: exp(energy)[m, n]  (f32r)
  av    psum [128, 258]: cols 0..255 = unnormalized out[n, (d,c)], col 256 = sum_m
  out_A [64, 8192]   : final (c, hw_local*4 + d), preloaded with x3d slice

All matmuls run as float32r (full-rate fp32 PE mode, even-N constraint).
"""

import sys
import numpy as np

sys.path.insert(0, "/opt/trn_rl_repo")

C = 64
D = 4
CQ = 8
H = 64
W = 64
N = H * W          # 4096 spatial positions per batch
NH = N // 2        # 2048 per core
KD = D * CQ        # 32  attention contraction channels
CD = D * C         # 256 attention value channels
NCORES = 8

_cache = {}


def _build_program():
    import concourse.bacc as bacc
    import concourse.bass as bass
    import concourse.mybir as mybir
    import concourse.tile as tile
    from contextlib import ExitStack

    F32 = mybir.dt.float32
    F32R = mybir.dt.float32r
    Exp = mybir.ActivationFunctionType.Exp
    ADD = mybir.AluOpType.add

    nc = bacc.Bacc("TRN2", target_bir_lowering=False)

    x_d = nc.dram_tensor("x", [C, N * D], F32R, kind="ExternalInput")
    x3_d = nc.dram_tensor("x3", [C, NH * D], F32, kind="ExternalInput")
    wqb_d = nc.dram_tensor("wqb", [C + 1, 512], F32R, kind="ExternalInput")
    wkb_d = nc.dram_tensor("wkb", [C + 1, 512], F32R, kind="ExternalInput")
    wv_d = nc.dram_tensor("wv", [C + 1, C], F32R, kind="ExternalInput")
    gm_d = nc.dram_tensor("gamma", [1, 1], F32, kind="ExternalInput")
    id_d = nc.dram_tensor("ident", [128, 128], F32, kind="ExternalInput")
    ones_d = nc.dram_tensor("ones", [1, N * D], F32R, kind="ExternalInput")
    out_d = nc.dram_tensor("out", [C, NH * D], F32, kind="ExternalOutput")

    with tile.TileContext(nc) as tc, ExitStack() as ctx:
        consts = ctx.enter_context(tc.tile_pool(name="consts", bufs=1))
        qkv = ctx.enter_context(tc.tile_pool(name="qkv", bufs=1))
        outp = ctx.enter_context(tc.tile_pool(name="outp", bufs=1))

        wqb = consts.tile([C + 1, 512], F32R)
        nc.sync.dma_start(out=wqb, in_=wqb_d[:, :])
        wkb = consts.tile([C + 1, 512], F32R)
        nc.sync.dma_start(out=wkb, in_=wkb_d[:, :])
        wv = consts.tile([C + 1, C], F32R)
        nc.sync.dma_start(out=wv, in_=wv_d[:, :])
        ident = consts.tile([128, 128], F32)
        nc.sync.dma_start(out=ident, in_=id_d[:, :])
        gam = consts.tile([128, 1], F32)
        nc.sync.dma_start(out=gam, in_=gm_d[:, :].partition_broadcast(128))

        k_sb = qkv.tile([128, N], F32R)
        q_sb = qkv.tile([128, NH], F32R)
        vt = qkv.tile([128, 32 * 258], F32R)
        vt3 = vt.rearrange("p (ch q) -> p ch q", q=258)  # [128, 32, 258]
        nc.sync.dma_start(
            out=vt3[:, :, 256:258],
            in_=bass.AP(ones_d, 0, [[0, 128], [1, 32], [1, 2]]),
        )
        out_A = outp.tile([C, NH * D], F32)
        nc.sync.dma_start(out=out_A, in_=x3_d[:, :])

        # ---------------- Phase A: QKV convs ----------------
        xa_stack = ExitStack()
        xapool = xa_stack.enter_context(tc.tile_pool(name="xa", bufs=1))
        xA = xapool.tile([C + 1, N * D], F32R)
        for w in range(8):
            nc.sync.dma_start(
                out=xA[0:C, w * 2048 : (w + 1) * 2048],
                in_=x_d[:, w * 2048 : (w + 1) * 2048],
            )
        nc.sync.dma_start(out=xA[C : C + 1, :], in_=ones_d[:, :])

        xa3 = xA.rearrange("p (n d) -> p n d", d=D)    # [65, 4096, 4]

        psKQ_stack = ExitStack()
        psKQ = psKQ_stack.enter_context(
            tc.tile_pool(name="psKQ", space="PSUM", bufs=2)
        )
        for w in range(8):
            k_ps = psKQ.tile([128, 512], F32, tag="kq_ps", name=f"k_ps_{w}")
            for d in range(D):
                nc.tensor.matmul(
                    k_ps,
                    wkb[:, d * 128 : (d + 1) * 128],
                    xa3[:, w * 512 : (w + 1) * 512, d],
                    start=(d == 0),
                    stop=(d == D - 1),
                )
            nc.vector.tensor_copy(out=k_sb[:, w * 512 : (w + 1) * 512], in_=k_ps)
        for w in range(4):
            q_ps = psKQ.tile([128, 512], F32, tag="kq_ps", name=f"q_ps_{w}")
            for d in range(D):
                nc.tensor.matmul(
                    q_ps,
                    wqb[:, d * 128 : (d + 1) * 128],
                    xa3[:, w * 512 : (w + 1) * 512, d],
                    start=(d == 0),
                    stop=(d == D - 1),
                )
            nc.vector.tensor_copy(out=q_sb[:, w * 512 : (w + 1) * 512], in_=q_ps)
        psKQ_stack.close()

        psE = ctx.enter_context(tc.tile_pool(name="psE", space="PSUM", bufs=2))

        psV_stack = ExitStack()
        psV = psV_stack.enter_context(
            tc.tile_pool(name="psV", space="PSUM", bufs=2)
        )
        for d in range(D):
            for g in range(8):
                v_ps = psV.tile([128, 256], F32, tag="v_ps", name=f"v_ps_{d}_{g}")
                for cc in range(4):
                    ch = g * 4 + cc
                    nc.tensor.matmul(
                        v_ps[:, cc * C : (cc + 1) * C],
                        xa3[:, ch * 128 : (ch + 1) * 128, d],
                        wv[:, :],
                        start=True,
                        stop=True,
                    )
                nc.vector.tensor_copy(
                    out=vt3[:, g * 4 : (g + 1) * 4, d * C : (d + 1) * C],
                    in_=v_ps.rearrange("p (cc o) -> p cc o", o=C),
                )
        psV_stack.close()
        xa_stack.close()

        # ---------------- Phase B: attention ----------------
        ptpool = ctx.enter_context(tc.tile_pool(name="pt", bufs=10))
        work = ctx.enter_context(tc.tile_pool(name="work", bufs=3))
        sm = ctx.enter_context(tc.tile_pool(name="sm", bufs=4))
        psAV = ctx.enter_context(tc.tile_pool(name="psAV", space="PSUM", bufs=2))
        psT = ctx.enter_context(tc.tile_pool(name="psT", space="PSUM", bufs=2))

        def emit_et_group(wi, g):
            """E_t + exp for m-chunks 4g..4g+3 of window wi -> one P_t group.
            The 4 chunk matmuls run concurrently in distinct PE row-groups
            (K=32 row tiling) against the 4 partition-replicas of k/q."""
            ptg = ptpool.tile([128, 2048], F32R, tag="ptg", name=f"ptg_{wi}_{g}")
            for hv in range(2):
                et = psE.tile([128, 1024], F32, tag="et", name=f"et_{wi}_{g}_{hv}")
                for j in range(2):
                    r = hv * 2 + j
                    ch = g * 4 + r
                    nc.tensor.matmul(
                        et[:, j * 512 : (j + 1) * 512],
                        k_sb[32 * r : 32 * (r + 1), ch * 128 : (ch + 1) * 128],
                        q_sb[32 * r : 32 * (r + 1), wi * 512 : (wi + 1) * 512],
                        start=True,
                        stop=True,
                        tile_position=(32 * r, 0),
                    )
                nc.scalar.activation(
                    out=ptg[:, hv * 1024 : (hv + 1) * 1024],
                    in_=et,
                    func=Exp,
                )
            return ptg

        def emit_av_block(wi, nb, groups):
            """attn @ [v|1] for n-block nb of window wi, normalize, transpose,
            accumulate into out_A (which holds x3d)."""
            av = psAV.tile([128, 258], F32, tag="av", name=f"av_{wi}_{nb}")
            for ch in range(32):
                g, o = divmod(ch, 4)
                nc.tensor.matmul(
                    av,
                    groups[g][:, o * 512 + nb * 128 : o * 512 + nb * 128 + 128],
                    vt[:, ch * 258 : (ch + 1) * 258],
                    start=(ch == 0),
                    stop=(ch == 31),
                )
            rc = sm.tile([128, 1], F32, tag="rc", name=f"rc_{wi}_{nb}")
            nc.vector.reciprocal(rc, av[:, 256:257])
            grc = sm.tile([128, 1], F32, tag="grc", name=f"grc_{wi}_{nb}")
            nc.vector.tensor_scalar_mul(grc, rc, gam)
            osb = work.tile([128, 256], F32, tag="osb", name=f"osb_{wi}_{nb}")
            nc.vector.tensor_scalar_mul(osb, av[:, 0:256], grc)
            tr = psT.tile([64, 512], F32, tag="tr", name=f"tr_{wi}_{nb}")
            for d in range(D):
                nc.tensor.transpose(
                    tr[:, d * 128 : (d + 1) * 128],
                    osb[:, d * C : (d + 1) * C],
                    ident,
                )
            hw0 = wi * 512 + nb * 128
            oslice = (
                out_A.rearrange("p (hw d) -> p hw d", d=D)[:, hw0 : hw0 + 128, :]
                .transpose([0, 2, 1])
            )  # [64, 4, 128] iterated (d, hw) to match tr
            tr3 = tr.rearrange("p (d nn) -> p d nn", nn=128)
            nc.vector.tensor_tensor(out=oslice, in0=tr3, in1=oslice, op=ADD)

        # software pipeline: E_t groups of window w interleave with AV of w-1
        prev_groups = None
        for w in range(4):
            groups = []
            for g in range(8):
                groups.append(emit_et_group(w, g))
                if prev_groups is not None and g % 2 == 1:
                    emit_av_block(w - 1, g // 2, prev_groups)
            if prev_groups is not None:
                wo = w - 1
                nc.sync.dma_start(
                    out=out_d[:, wo * 2048 : (wo + 1) * 2048],
                    in_=out_A[:, wo * 2048 : (wo + 1) * 2048],
                )
            prev_groups = groups
        for nb in range(4):
            emit_av_block(3, nb, prev_groups)
        nc.sync.dma_start(out=out_d[:, 3 * 2048 :], in_=out_A[:, 3 * 2048 :])

    nc.compile()
    return nc


def _get_program():
    if "nc" not in _cache:
        _cache["nc"] = _build_program()
    return _cache["nc"]


def _host_weights(Wq, bq, Wk, bk, Wv, bv):
    """Blocked + replicated qk conv weights.

    lhsT slice [:, d*128:(d+1)*128] maps x_aug (65 rows: 64 channels +
    ones row) to psum partitions r*32 + (d*8+cq) for all 4 replicas r,
    with zero columns for other d (the 4 d-matmuls accumulate)."""
    wqb = np.zeros((C + 1, 512), np.float32)
    wkb = np.zeros((C + 1, 512), np.float32)
    for d in range(D):
        for r in range(4):
            for cq in range(CQ):
                col = d * 128 + r * 32 + d * CQ + cq
                wqb[0:C, col] = Wq[cq, :]
                wqb[C, col] = bq[cq]
                wkb[0:C, col] = Wk[cq, :]
                wkb[C, col] = bk[cq]
    wv_aug = np.concatenate([Wv.T, bv[None, :]], axis=0).astype(np.float32)
    return wqb, wkb, np.ascontiguousarray(wv_aug)


def _run(inputs, trace=False):
    from concourse.bass_utils import run_bass_kernel_spmd

    x2d = np.asarray(inputs["x2d"], np.float32)
    x3d = np.asarray(inputs["x3d"], np.float32)
    wqb, wkb, wv_aug = _host_weights(
        np.asarray(inputs["Wq"], np.float32), np.asarray(inputs["bq"], np.float32),
        np.asarray(inputs["Wk"], np.float32), np.asarray(inputs["bk"], np.float32),
        np.asarray(inputs["Wv"], np.float32), np.asarray(inputs["bv"], np.float32),
    )
    gamma = np.asarray(inputs["gamma"], np.float32).reshape(1, 1)
    ident = np.eye(128, dtype=np.float32)
    ones = np.ones((1, N * D), np.float32)

    in_maps = []
    for core in range(NCORES):
        b, half = divmod(core, 2)
        xb3 = x2d[b].reshape(C, N, D)
        if half:
            xb3 = np.concatenate([xb3[:, NH:], xb3[:, :NH]], axis=1)
        lo, hi = half * NH * D, (half + 1) * NH * D
        in_maps.append({
            "x": np.ascontiguousarray(xb3.reshape(C, N * D)),
            "x3": np.ascontiguousarray(x3d[b].reshape(C, N * D)[:, lo:hi]),
            "wqb": wqb,
            "wkb": wkb,
            "wv": wv_aug,
            "gamma": gamma,
            "ident": ident,
            "ones": ones,
        })

    nc = _get_program()
    res = run_bass_kernel_spmd(
        nc, in_maps, core_ids=list(range(NCORES)), trace=trace
    )

    out_full = np.empty((4, C, H, W, D), np.float32)
    for core in range(NCORES):
        b, half = divmod(core, 2)
        o = res.results[core]["out"].reshape(C, H // 2, W, D)
        out_full[b, :, half * (H // 2) : (half + 1) * (H // 2), :, :] = o
    return out_full, res


def kernel(**inputs):
    out, _ = _run(inputs, trace=False)
    return out


# revision 9
# speedup vs baseline: 1.7491x; 1.1446x over previous
"""Trainium2 Bass kernel for nn_CBAM_83691732730338.

Self-attention block (HWxHW attention over (C,D)-channels) + residual:
  x = transpose(x2d)                        # (B, C, D, H, W)
  q/k/v = 1x1 conv over C (collapsed to channel matmuls, D folded into
          the attention channel dim), N = H*W
  energy = q^T k  (per batch, N x N), attn = softmax(energy, axis=-1)
  out = v @ attn^T ; out = gamma*out + x3d

Sharding: 8 cores = 4 batches x 2 spatial halves. Attention is invariant
to a permutation of the softmax/value positions m, so each core receives
its batch's x ROTATED so that the core's n-half sits at positions
0..2047: q is computed from positions 0..2047, k/v over all 4096, and
the program is identical on every core (SPMD) with no runtime offsets.

Kernel-internal layouts (per core):
  xA    [65, 16384]  : rot(x[b]) as (c, hw*D + d), row 64 = 1.0 (bias trick)
  k_sb  [128, 4096]  : k[(d*8+cq), m] replicated 4x along partitions
                       (row r*32 + dq) -- feeds 4x row-tiled energy matmuls
  q_sb  [128, 2048]  : q likewise, n = local 0..2047
  vt    [128, 32*258]: chunk-major v^T; cols [ch*258 + d*64 + c] = v[(d,c), m],
                       cols ch*258+{256,257} = 1.0 (row-sum trick + even pad)
  P_t   [128, 2048]x8 groups per window: exp(energy)[m, n]  (f32r)
  av    psum [128, 258]: cols 0..255 = unnormalized out[n, (d,c)], col 256 = sum_m
  out_A [64, 8192]   : final (c, hw_local*4 + d), preloaded with x3d slice

All matmuls run as float32r (full-rate fp32 PE mode, even-N constraint).
"""

import sys
import numpy as np

sys.path.insert(0, "/opt/trn_rl_repo")

C = 64
D = 4
CQ = 8
H = 64
W = 64
N = H * W          # 4096 spatial positions per batch
NH = N // 2        # 2048 per core
KD = D * CQ        # 32  attention contraction channels
CD = D * C         # 256 attention value channels
NCORES = 8

_cache = {}


def _build_program():
    import concourse.bacc as bacc
    import concourse.bass as bass
    import concourse.mybir as mybir
    import concourse.tile as tile
    from contextlib import ExitStack

    F32 = mybir.dt.float32
    F32R = mybir.dt.float32r
    F16 = mybir.dt.float16
    Exp = mybir.ActivationFunctionType.Exp
    ADD = mybir.AluOpType.add

    nc = bacc.Bacc("TRN2", target_bir_lowering=False)

    x_d = nc.dram_tensor("x", [C, N * D], F16, kind="ExternalInput")
    x3_d = nc.dram_tensor("x3", [C, NH * D], F32, kind="ExternalInput")
    wqb_d = nc.dram_tensor("wqb", [C + 1, 512], F16, kind="ExternalInput")
    wkb_d = nc.dram_tensor("wkb", [C + 1, 512], F16, kind="ExternalInput")
    wv_d = nc.dram_tensor("wv", [C + 1, C], F16, kind="ExternalInput")
    gm_d = nc.dram_tensor("gamma", [1, 1], F32, kind="ExternalInput")
    id_d = nc.dram_tensor("ident", [128, 128], F32, kind="ExternalInput")
    ones_d = nc.dram_tensor("ones", [1, N * D], F16, kind="ExternalInput")
    onesr_d = nc.dram_tensor("onesr", [1, 64], F32R, kind="ExternalInput")
    out_d = nc.dram_tensor("out", [C, NH * D], F32, kind="ExternalOutput")

    with tile.TileContext(nc) as tc, ExitStack() as ctx:
        consts = ctx.enter_context(tc.tile_pool(name="consts", bufs=1))
        qkv = ctx.enter_context(tc.tile_pool(name="qkv", bufs=1))
        outp = ctx.enter_context(tc.tile_pool(name="outp", bufs=1))

        # x windows + qk weights first on the sync HWDGE queue so the
        # first conv matmul can start ASAP; everything else later / on the
        # scalar queue.
        xa_stack = ExitStack()
        xapool = xa_stack.enter_context(tc.tile_pool(name="xa", bufs=1))
        xA = xapool.tile([C + 1, N * D], F16)
        nc.sync.dma_start(out=xA[0:C, 0:2048], in_=x_d[:, 0:2048])
        nc.sync.dma_start(out=xA[C : C + 1, :], in_=ones_d[:, :])
        wqb = consts.tile([C + 1, 512], F16)
        wkb = consts.tile([C + 1, 512], F16)
        nc.sync.dma_start(out=wkb, in_=wkb_d[:, :])
        nc.sync.dma_start(out=wqb, in_=wqb_d[:, :])
        for w in range(1, 8):
            eng = nc.sync if w % 2 else nc.scalar
            eng.dma_start(
                out=xA[0:C, w * 2048 : (w + 1) * 2048],
                in_=x_d[:, w * 2048 : (w + 1) * 2048],
            )
        wv = consts.tile([C + 1, C], F16)
        nc.sync.dma_start(out=wv, in_=wv_d[:, :])
        ident = consts.tile([128, 128], F32)
        nc.scalar.dma_start(out=ident, in_=id_d[:, :])
        gam = consts.tile([128, 1], F32)
        nc.scalar.dma_start(out=gam, in_=gm_d[:, :].partition_broadcast(128))

        k_sb = qkv.tile([128, N], F32R)
        q_sb = qkv.tile([128, NH], F32R)
        vt = qkv.tile([128, 32 * 258], F32R)
        vt3 = vt.rearrange("p (ch q) -> p ch q", q=258)  # [128, 32, 258]
        nc.scalar.dma_start(
            out=vt3[:, :, 256:258],
            in_=bass.AP(onesr_d, 0, [[0, 128], [1, 32], [1, 2]]),
        )
        out_A = outp.tile([C, NH * D], F32)
        nc.scalar.dma_start(out=out_A, in_=x3_d[:, :])

        # ---------------- Phase A: QKV convs ----------------

        xa3 = xA.rearrange("p (n d) -> p n d", d=D)    # [65, 4096, 4]

        psKQ_stack = ExitStack()
        psKQ = psKQ_stack.enter_context(
            tc.tile_pool(name="psKQ", space="PSUM", bufs=2)
        )
        for w in range(8):
            k_ps = psKQ.tile([128, 512], F32, tag="kq_ps", name=f"k_ps_{w}")
            for d in range(D):
                nc.tensor.matmul(
                    k_ps,
                    wkb[:, d * 128 : (d + 1) * 128],
                    xa3[:, w * 512 : (w + 1) * 512, d],
                    start=(d == 0),
                    stop=(d == D - 1),
                )
            nc.vector.tensor_copy(out=k_sb[:, w * 512 : (w + 1) * 512], in_=k_ps)
        for w in range(4):
            q_ps = psKQ.tile([128, 512], F32, tag="kq_ps", name=f"q_ps_{w}")
            for d in range(D):
                nc.tensor.matmul(
                    q_ps,
                    wqb[:, d * 128 : (d + 1) * 128],
                    xa3[:, w * 512 : (w + 1) * 512, d],
                    start=(d == 0),
                    stop=(d == D - 1),
                )
            nc.vector.tensor_copy(out=q_sb[:, w * 512 : (w + 1) * 512], in_=q_ps)
        psKQ_stack.close()

        psE = ctx.enter_context(tc.tile_pool(name="psE", space="PSUM", bufs=2))

        psV_stack = ExitStack()
        psV = psV_stack.enter_context(
            tc.tile_pool(name="psV", space="PSUM", bufs=2)
        )
        for d in range(D):
            for g in range(8):
                v_ps = psV.tile([128, 256], F32, tag="v_ps", name=f"v_ps_{d}_{g}")
                for cc in range(4):
                    ch = g * 4 + cc
                    nc.tensor.matmul(
                        v_ps[:, cc * C : (cc + 1) * C],
                        xa3[:, ch * 128 : (ch + 1) * 128, d],
                        wv[:, :],
                        start=True,
                        stop=True,
                    )
                nc.vector.tensor_copy(
                    out=vt3[:, g * 4 : (g + 1) * 4, d * C : (d + 1) * C],
                    in_=v_ps.rearrange("p (cc o) -> p cc o", o=C),
                )
        psV_stack.close()
        xa_stack.close()

        # ---------------- Phase B: attention ----------------
        ptpool = ctx.enter_context(tc.tile_pool(name="pt", bufs=10))
        work = ctx.enter_context(tc.tile_pool(name="work", bufs=3))
        sm = ctx.enter_context(tc.tile_pool(name="sm", bufs=4))
        psAV = ctx.enter_context(tc.tile_pool(name="psAV", space="PSUM", bufs=2))
        psT = ctx.enter_context(tc.tile_pool(name="psT", space="PSUM", bufs=2))

        def emit_et_group(wi, g):
            """E_t + exp for m-chunks 4g..4g+3 of window wi -> one P_t group.
            The 4 chunk matmuls run concurrently in distinct PE row-groups
            (K=32 row tiling) against the 4 partition-replicas of k/q."""
            ptg = ptpool.tile([128, 2048], F32R, tag="ptg", name=f"ptg_{wi}_{g}")
            for hv in range(2):
                et = psE.tile([128, 1024], F32, tag="et", name=f"et_{wi}_{g}_{hv}")
                for j in range(2):
                    r = hv * 2 + j
                    ch = g * 4 + r
                    nc.tensor.matmul(
                        et[:, j * 512 : (j + 1) * 512],
                        k_sb[32 * r : 32 * (r + 1), ch * 128 : (ch + 1) * 128],
                        q_sb[32 * r : 32 * (r + 1), wi * 512 : (wi + 1) * 512],
                        start=True,
                        stop=True,
                        tile_position=(32 * r, 0),
                    )
                nc.scalar.activation(
                    out=ptg[:, hv * 1024 : (hv + 1) * 1024],
                    in_=et,
                    func=Exp,
                )
            return ptg

        def emit_av_mm(wi, nb, groups):
            """attn @ [v|1] matmuls + normalization for n-block nb."""
            av = psAV.tile([128, 258], F32, tag="av", name=f"av_{wi}_{nb}")
            for ch in range(32):
                g, o = divmod(ch, 4)
                nc.tensor.matmul(
                    av,
                    groups[g][:, o * 512 + nb * 128 : o * 512 + nb * 128 + 128],
                    vt[:, ch * 258 : (ch + 1) * 258],
                    start=(ch == 0),
                    stop=(ch == 31),
                )
            rc = sm.tile([128, 1], F32, tag="rc", name=f"rc_{wi}_{nb}")
            nc.vector.reciprocal(rc, av[:, 256:257])
            grc = sm.tile([128, 1], F32, tag="grc", name=f"grc_{wi}_{nb}")
            nc.vector.tensor_scalar_mul(grc, rc, gam)
            osb = work.tile([128, 256], F32, tag="osb", name=f"osb_{wi}_{nb}")
            nc.vector.tensor_scalar_mul(osb, av[:, 0:256], grc)
            return osb

        def emit_av_finish(wi, nb, osb):
            """PE-transpose the normalized block and add into out_A."""
            tr = psT.tile([64, 512], F32, tag="tr", name=f"tr_{wi}_{nb}")
            for d in range(D):
                nc.tensor.transpose(
                    tr[:, d * 128 : (d + 1) * 128],
                    osb[:, d * C : (d + 1) * C],
                    ident,
                )
            hw0 = wi * 512 + nb * 128
            oslice = (
                out_A.rearrange("p (hw d) -> p hw d", d=D)[:, hw0 : hw0 + 128, :]
                .transpose([0, 2, 1])
            )  # [64, 4, 128] iterated (d, hw) to match tr
            tr3 = tr.rearrange("p (d nn) -> p d nn", nn=128)
            nc.vector.tensor_tensor(out=oslice, in0=tr3, in1=oslice, op=ADD)

        # software pipeline: E_t groups of window w interleave with AV of
        # w-1; each AV block's transpose+add trails by one unit so the PE
        # never waits on the DVE normalize.
        prev_groups = None
        pending = []          # (wi, nb, osb) awaiting transpose+add
        done_adds = [0, 0, 0, 0]

        def flush_pending():
            while pending:
                pwi, pnb, posb = pending.pop(0)
                emit_av_finish(pwi, pnb, posb)
                done_adds[pwi] += 1
                if done_adds[pwi] == 4:
                    nc.sync.dma_start(
                        out=out_d[:, pwi * 2048 : (pwi + 1) * 2048],
                        in_=out_A[:, pwi * 2048 : (pwi + 1) * 2048],
                    )

        for w in range(4):
            groups = []
            for g in range(8):
                groups.append(emit_et_group(w, g))
                if prev_groups is not None and g % 2 == 1:
                    flush_pending()
                    pending.append(
                        (w - 1, g // 2, emit_av_mm(w - 1, g // 2, prev_groups))
                    )
            prev_groups = groups
        for nb in range(4):
            flush_pending()
            pending.append((3, nb, emit_av_mm(3, nb, prev_groups)))
        flush_pending()

    nc.compile()
    return nc


def _get_program():
    if "nc" not in _cache:
        _cache["nc"] = _build_program()
    return _cache["nc"]


def _host_weights(Wq, bq, Wk, bk, Wv, bv):
    """Blocked + replicated qk conv weights.

    lhsT slice [:, d*128:(d+1)*128] maps x_aug (65 rows: 64 channels +
    ones row) to psum partitions r*32 + (d*8+cq) for all 4 replicas r,
    with zero columns for other d (the 4 d-matmuls accumulate)."""
    wqb = np.zeros((C + 1, 512), np.float32)
    wkb = np.zeros((C + 1, 512), np.float32)
    for d in range(D):
        for r in range(4):
            for cq in range(CQ):
                col = d * 128 + r * 32 + d * CQ + cq
                wqb[0:C, col] = Wq[cq, :]
                wqb[C, col] = bq[cq]
                wkb[0:C, col] = Wk[cq, :]
                wkb[C, col] = bk[cq]
    wv_aug = np.concatenate([Wv.T, bv[None, :]], axis=0).astype(np.float32)
    return wqb, wkb, np.ascontiguousarray(wv_aug)


def _run(inputs, trace=False):
    from concourse.bass_utils import run_bass_kernel_spmd

    x2d = np.asarray(inputs["x2d"], np.float32)
    x3d = np.asarray(inputs["x3d"], np.float32)
    wqb, wkb, wv_aug = _host_weights(
        np.asarray(inputs["Wq"], np.float32), np.asarray(inputs["bq"], np.float32),
        np.asarray(inputs["Wk"], np.float32), np.asarray(inputs["bk"], np.float32),
        np.asarray(inputs["Wv"], np.float32), np.asarray(inputs["bv"], np.float32),
    )
    gamma = np.asarray(inputs["gamma"], np.float32).reshape(1, 1)
    ident = np.eye(128, dtype=np.float32)
    ones = np.ones((1, N * D), np.float16)
    onesr = np.ones((1, 64), np.float32)
    wqb = wqb.astype(np.float16)
    wkb = wkb.astype(np.float16)
    wv_aug = wv_aug.astype(np.float16)

    in_maps = []
    for core in range(NCORES):
        b, half = divmod(core, 2)
        xb3 = x2d[b].reshape(C, N, D)
        if half:
            xb3 = np.concatenate([xb3[:, NH:], xb3[:, :NH]], axis=1)
        lo, hi = half * NH * D, (half + 1) * NH * D
        in_maps.append({
            "x": np.ascontiguousarray(xb3.reshape(C, N * D).astype(np.float16)),
            "x3": np.ascontiguousarray(x3d[b].reshape(C, N * D)[:, lo:hi]),
            "wqb": wqb,
            "wkb": wkb,
            "wv": wv_aug,
            "gamma": gamma,
            "ident": ident,
            "ones": ones,
            "onesr": onesr,
        })

    nc = _get_program()
    res = run_bass_kernel_spmd(
        nc, in_maps, core_ids=list(range(NCORES)), trace=trace
    )

    out_full = np.empty((4, C, H, W, D), np.float32)
    for core in range(NCORES):
        b, half = divmod(core, 2)
        o = res.results[core]["out"].reshape(C, H // 2, W, D)
        out_full[b, :, half * (H // 2) : (half + 1) * (H // 2), :, :] = o
    return out_full, res


def kernel(**inputs):
    out, _ = _run(inputs, trace=False)
    return out
